# revision 25
# baseline (speedup 1.0000x reference)
"""Trainium2 Bass kernel for nn_LinkPredictor.

Reference computation (B=4, N=256, T=16, F=128, H=256):
    h = mean_T(nodefeat)                      # [B,N,F]
    a = h @ W1[:, :F].T                       # [B,N,H]
    c = h @ W1[:, F:].T                       # [B,N,H]
    logits[b,i,j] = W2[0] . relu(a[b,i] + c[b,j] + b1) + b2   # [B,N,N]

Sharding: 8 cores; core k handles batch b=k//2, i-half k%2 (128 i-rows x
256 j-cols of one batch's NxN grid).  Each core only needs nodefeat[b],
since the pairwise grid never mixes batches.

Per-core device plan (all layouts h/f-on-partitions):
  - hT[f, j] via 32 PE matmuls: stationary = bf16 nodefeat octet
    [(j8,t16)=128p, f], moving = selection matrix S [128, 8] with
    S[(j8,t), m] = (j8==m)/16; PSUM accumulates in fp32.
  - cT/aT via fp32 matmuls with W1 slices as stationary; b1 folded into
    aT; cT stored bf16.
  - Pairwise: for each i (128) and h-tile (2): act = relu(cT + aT[:,i])
    in bf16, split across VectorE (tensor_scalar add+max, 2x bf16),
    ScalarE (activation Relu, per-partition bias) and GpSimd.
  - Reduction over h on TensorE: act is the bf16 moving operand (1
    cyc/col); stationary = [128, 32] zero-padded bf16 w2 column r=i%32;
    each 32-i group PSUM-accumulates into its own bank and is drained
    (+b2) as the next group runs.
"""

import os
import sys

import numpy as np

_B, _N, _T, _F, _H = 4, 256, 16, 128, 256
_NCORES = 8
_NF_CHUNKS = 4  # nf DMA split for load/compute overlap

_CACHE = {}


def _ensure_paths():
    for p in (
        "/root/.axon_site",
        "/root/.axon_site/_ro/trn_rl_repo",
        "/root/.axon_site/_ro/pypackages",
        "/opt/trn_rl_repo",
    ):
        if os.path.isdir(p) and p not in sys.path:
            sys.path.append(p)


def build_nc():
    """Build the per-core Bass program (same program for all 8 cores)."""
    _ensure_paths()
    import concourse.mybir as mybir
    import concourse.tile as tile
    from concourse import bacc

    f32 = mybir.dt.float32
    bf16 = mybir.dt.bfloat16
    Alu = mybir.AluOpType
    Act = mybir.ActivationFunctionType

    nc = bacc.Bacc("TRN2", target_bir_lowering=False, debug=False)

    nf = nc.declare_dram_parameter("nf", [128, 32, 128], bf16, isOutput=False)
    smat = nc.declare_dram_parameter("smat", [128, 8], bf16, isOutput=False)
    w1at = nc.declare_dram_parameter("w1at", [128, 2, 128], f32, isOutput=False)
    w1ct = nc.declare_dram_parameter("w1ct", [128, 2, 128], f32, isOutput=False)
    b1t = nc.declare_dram_parameter("b1t", [128, 2], f32, isOutput=False)
    w2b = nc.declare_dram_parameter("w2b", [128, 2, 32, 32], bf16, isOutput=False)
    b2c = nc.declare_dram_parameter("b2c", [128, 1], f32, isOutput=False)
    outd = nc.declare_dram_parameter("out", [4, 32, 256], f32, isOutput=True)

    with tile.TileContext(nc) as tc:
        with (
            tc.tile_pool(name="const", bufs=1) as constp,
            tc.tile_pool(name="data", bufs=1) as datap,
            tc.tile_pool(name="act", bufs=15) as actp,
            tc.tile_pool(name="ph", bufs=1, space="PSUM") as php,
            tc.tile_pool(name="pc", bufs=1, space="PSUM") as pcp,
            tc.tile_pool(name="pl", bufs=4, space="PSUM") as plp,
        ):
            # DMAs ordered by when the data is needed.
            smat_sb = constp.tile([128, 8], bf16, tag="smat")
            nc.sync.dma_start(out=smat_sb[:], in_=smat[:])
            nf_sb = constp.tile([128, 32, 128], bf16, tag="nf")
            ocpc = 32 // _NF_CHUNKS  # octets per chunk
            for ch in range(_NF_CHUNKS):
                o0 = ch * ocpc
                nc.gpsimd.dma_start(
                    out=nf_sb[:, o0 : o0 + ocpc, :], in_=nf[:, o0 : o0 + ocpc, :]
                )
            w1ct_sb = constp.tile([128, 2, 128], f32, tag="w1ct")
            nc.sync.dma_start(out=w1ct_sb[:], in_=w1ct[:])
            w1at_sb = constp.tile([128, 2, 128], f32, tag="w1at")
            nc.sync.dma_start(out=w1at_sb[:], in_=w1at[:])
            b1t_sb = constp.tile([128, 2], f32, tag="b1t")
            nc.sync.dma_start(out=b1t_sb[:], in_=b1t[:])
            w2b_sb = constp.tile([128, 2, 32, 32], bf16, tag="w2b")
            nc.sync.dma_start(out=w2b_sb[:], in_=w2b[:])
            b2c_sb = constp.tile([128, 1], f32, tag="b2c")
            nc.sync.dma_start(out=b2c_sb[:], in_=b2c[:])

            # hT[f, j] = mean over T, via per-octet matmuls against S.
            ph = php.tile([128, 256], f32, tag="ph")
            for o in range(32):
                nc.tensor.matmul(
                    ph[:, 8 * o : 8 * o + 8],
                    lhsT=nf_sb[:, o, :],
                    rhs=smat_sb[:],
                    start=True,
                    stop=True,
                )
            hT = datap.tile([128, 256], f32, tag="hT")
            nc.vector.tensor_copy(hT[:], ph[:])

            # cT[h, j] (bf16) and aT[h, i]+b1 (fp32) for both h-tiles.
            # aTb4 stores each bias column replicated x4 so that every
            # column sits at a 16-byte offset — the VE bf16 dual-op
            # tensor_scalar falls off a microcode cliff (280ns -> 1.2-3.9us)
            # when the per-partition scalar pointer is not 16B-aligned.
            cT = [datap.tile([128, 256], bf16, tag=f"cT{t}", name=f"cT{t}") for t in range(2)]
            aTb4 = [datap.tile([128, 128, 4], f32, tag=f"aTb4{t}", name=f"aTb4{t}") for t in range(2)]
            for t in range(2):
                pc = pcp.tile([128, 256], f32, tag="pc")
                nc.tensor.matmul(
                    pc[:], lhsT=w1ct_sb[:, t, :], rhs=hT[:], start=True, stop=True
                )
                nc.scalar.copy(cT[t][:], pc[:])
                pa = pcp.tile([128, 128], f32, tag="pa")
                nc.tensor.matmul(
                    pa[:], lhsT=w1at_sb[:, t, :], rhs=hT[:, 0:128], start=True, stop=True
                )
                nc.vector.tensor_scalar(
                    aTb4[t][:, :, :],
                    pa[:].broadcast_to([128, 128, 4]),
                    b1t_sb[:, t : t + 1],
                    None,
                    Alu.add,
                )

            # Pairwise: accumulate each 32-i group into its own PSUM bank,
            # drain groups as they complete.  Act tiles split VE/SE/GP.
            pl = None
            for i in range(128):
                g, r = divmod(i, 32)
                if r == 0:
                    pl = plp.tile([32, 256], f32, tag="pl", name=f"pl{g}")
                for t in range(2):
                    idx = 2 * i + t
                    a_col = aTb4[t][:, i, 0:1]
                    if idx % 3 == 1:
                        asb = actp.tile([128, 256], bf16, tag="acts")
                        nc.scalar.activation(asb[:], cT[t][:], Act.Relu, bias=a_col)
                        mv = asb
                    else:
                        av = actp.tile([128, 256], bf16, tag="actv")
                        nc.vector.tensor_scalar(
                            av[:], cT[t][:], a_col, 0.0, Alu.add, Alu.max
                        )
                        mv = av
                    nc.tensor.matmul(
                        pl[:, :],
                        lhsT=w2b_sb[:, t, r, :],
                        rhs=mv[:],
                        start=(r == 0 and t == 0),
                        stop=(r == 31 and t == 1),
                    )
                if r == 31:
                    osb = datap.tile([32, 256], f32, tag=f"osb{g}", name=f"osb{g}")
                    nc.vector.tensor_scalar(
                        osb[:], pl[:, :], b2c_sb[0:32, :], None, Alu.add
                    )
                    nc.sync.dma_start(out=outd[g], in_=osb[:])

    nc.compile()
    return nc


def make_in_maps(nodefeat, W1, b1, W2, b2):
    """Host-side sharding/layout prep."""
    import ml_dtypes

    bf16 = ml_dtypes.bfloat16
    nodefeat = np.asarray(nodefeat, dtype=np.float32)
    W1 = np.asarray(W1, dtype=np.float32)
    b1 = np.asarray(b1, dtype=np.float32)
    W2 = np.asarray(W2, dtype=np.float32)
    b2 = np.asarray(b2, dtype=np.float32)

    smat = (np.repeat(np.eye(8, dtype=np.float32), 16, axis=0) / 16.0).astype(bf16)

    W1a, W1c = W1[:, :_F], W1[:, _F:]
    w1at = np.ascontiguousarray(np.stack([W1a[:128].T, W1a[128:].T], axis=1))
    w1ct = np.ascontiguousarray(np.stack([W1c[:128].T, W1c[128:].T], axis=1))
    b1t = np.ascontiguousarray(b1.reshape(2, 128).T)

    w2r = W2[0].reshape(2, 128)  # [ht, p]
    w2b = np.zeros((128, 2, 32, 32), dtype=np.float32)
    idx = np.arange(32)
    w2b[:, :, idx, idx] = w2r.T[:, :, None]
    w2b = w2b.astype(bf16)

    b2c = np.full((128, 1), b2[0], dtype=np.float32)

    in_maps = []
    for k in range(_NCORES):
        b, ih = divmod(k, 2)
        nf_b = nodefeat[b]  # [256, 16, 128]
        if ih:
            nf_b = np.concatenate([nf_b[128:], nf_b[:128]], axis=0)
        # [256,16,128] -> [32 oct, (j8,t16)=128, 128 f] -> [128, 32, 128]
        nf_dev = np.ascontiguousarray(
            nf_b.reshape(32, 128, 128).transpose(1, 0, 2).astype(bf16)
        )
        in_maps.append(
            {
                "nf": nf_dev,
                "smat": smat,
                "w1at": w1at,
                "w1ct": w1ct,
                "b1t": b1t,
                "w2b": w2b,
                "b2c": b2c,
            }
        )
    return in_maps


def assemble_output(results):
    out = np.empty((_B, _N, _N), dtype=np.float32)
    for k in range(_NCORES):
        b, ih = divmod(k, 2)
        r = results[k]["out"].reshape(128, 256)  # [i, j] (j core-local order)
        if ih:
            r = np.concatenate([r[:, 128:], r[:, :128]], axis=1)
        out[b, ih * 128 : (ih + 1) * 128, :] = r
    return out


def _get_nc():
    if "nc" not in _CACHE:
        _CACHE["nc"] = build_nc()
    return _CACHE["nc"]


def kernel(nodefeat, W1, b1, W2, b2):
    _ensure_paths()
    from concourse.bass_utils import run_bass_kernel_spmd

    nc = _get_nc()
    in_maps = make_in_maps(nodefeat, W1, b1, W2, b2)
    res = run_bass_kernel_spmd(nc, in_maps, list(range(_NCORES)))
    return assemble_output(res.results)


# revision 26
# speedup vs baseline: 1.0382x; 1.0382x over previous
"""Trainium2 Bass kernel for nn_LinkPredictor.

Reference computation (B=4, N=256, T=16, F=128, H=256):
    h = mean_T(nodefeat)                      # [B,N,F]
    a = h @ W1[:, :F].T                       # [B,N,H]
    c = h @ W1[:, F:].T                       # [B,N,H]
    logits[b,i,j] = W2[0] . relu(a[b,i] + c[b,j] + b1) + b2   # [B,N,N]

Sharding: 8 cores; core k handles batch b=k//2, i-half k%2 (128 i-rows x
256 j-cols of one batch's NxN grid).  Each core only needs nodefeat[b],
since the pairwise grid never mixes batches.

Per-core device plan (all layouts h/f-on-partitions):
  - hT[f, j] via 32 PE matmuls: stationary = bf16 nodefeat octet
    [(j8,t16)=128p, f], moving = selection matrix S [128, 8] with
    S[(j8,t), m] = (j8==m)/16; PSUM accumulates in fp32.
  - cT/aT via fp32 matmuls with W1 slices as stationary; b1 folded into
    aT; cT stored bf16.
  - Pairwise: for each i (128) and h-tile (2): act = relu(cT + aT[:,i])
    in bf16, split ~2:1 across VectorE (dual-op tensor_scalar add+max,
    bf16 fast path needs the 16B-aligned bias column from aTb4) and
    ScalarE (activation Relu, per-partition bias).  GpSimd measured
    10-100x slower for these ops and is not used.
  - Reduction over h on TensorE: act is the bf16 moving operand (1
    cyc/col); stationary = [128, 32] zero-padded bf16 w2 column r=i%32;
    each 32-i group PSUM-accumulates into its own bank and is drained
    (+b2) as the next group runs.
"""

import os
import sys

import numpy as np

_B, _N, _T, _F, _H = 4, 256, 16, 128, 256
_NCORES = 8
_NF_CHUNKS = 4  # nf DMA split for load/compute overlap

_CACHE = {}


def _ensure_paths():
    for p in (
        "/root/.axon_site",
        "/root/.axon_site/_ro/trn_rl_repo",
        "/root/.axon_site/_ro/pypackages",
        "/opt/trn_rl_repo",
    ):
        if os.path.isdir(p) and p not in sys.path:
            sys.path.append(p)


def build_nc():
    """Build the per-core Bass program (same program for all 8 cores)."""
    _ensure_paths()
    import concourse.mybir as mybir
    import concourse.tile as tile
    from concourse import bacc

    f32 = mybir.dt.float32
    bf16 = mybir.dt.bfloat16
    Alu = mybir.AluOpType
    Act = mybir.ActivationFunctionType

    nc = bacc.Bacc("TRN2", target_bir_lowering=False, debug=False)

    nf = nc.declare_dram_parameter("nf", [128, 32, 128], bf16, isOutput=False)
    smat = nc.declare_dram_parameter("smat", [128, 8], bf16, isOutput=False)
    w1at = nc.declare_dram_parameter("w1at", [128, 2, 128], f32, isOutput=False)
    w1ct = nc.declare_dram_parameter("w1ct", [128, 2, 128], f32, isOutput=False)
    b1t = nc.declare_dram_parameter("b1t", [128, 2], f32, isOutput=False)
    w2b = nc.declare_dram_parameter("w2b", [128, 2, 32, 32], bf16, isOutput=False)
    b2c = nc.declare_dram_parameter("b2c", [128, 1], f32, isOutput=False)
    outd = nc.declare_dram_parameter("out", [4, 32, 256], f32, isOutput=True)

    with tile.TileContext(nc) as tc:
        with (
            tc.tile_pool(name="const", bufs=1) as constp,
            tc.tile_pool(name="data", bufs=1) as datap,
            tc.tile_pool(name="act", bufs=15) as actp,
            tc.tile_pool(name="ph", bufs=1, space="PSUM") as php,
            tc.tile_pool(name="pc", bufs=1, space="PSUM") as pcp,
            tc.tile_pool(name="pl", bufs=4, space="PSUM") as plp,
        ):
            # DMAs ordered by when the data is needed.
            smat_sb = constp.tile([128, 8], bf16, tag="smat")
            nc.sync.dma_start(out=smat_sb[:], in_=smat[:])
            nf_sb = constp.tile([128, 32, 128], bf16, tag="nf")
            ocpc = 32 // _NF_CHUNKS  # octets per chunk
            for ch in range(_NF_CHUNKS):
                o0 = ch * ocpc
                nc.sync.dma_start(
                    out=nf_sb[:, o0 : o0 + ocpc, :], in_=nf[:, o0 : o0 + ocpc, :]
                )
            w1ct_sb = constp.tile([128, 2, 128], f32, tag="w1ct")
            nc.sync.dma_start(out=w1ct_sb[:], in_=w1ct[:])
            w1at_sb = constp.tile([128, 2, 128], f32, tag="w1at")
            nc.sync.dma_start(out=w1at_sb[:], in_=w1at[:])
            b1t_sb = constp.tile([128, 2], f32, tag="b1t")
            nc.sync.dma_start(out=b1t_sb[:], in_=b1t[:])
            w2b_sb = constp.tile([128, 2, 32, 32], bf16, tag="w2b")
            nc.sync.dma_start(out=w2b_sb[:], in_=w2b[:])
            b2c_sb = constp.tile([128, 1], f32, tag="b2c")
            nc.sync.dma_start(out=b2c_sb[:], in_=b2c[:])

            # hT[f, j] = mean over T, via per-octet matmuls against S.
            ph = php.tile([128, 256], f32, tag="ph")
            for o in range(32):
                nc.tensor.matmul(
                    ph[:, 8 * o : 8 * o + 8],
                    lhsT=nf_sb[:, o, :],
                    rhs=smat_sb[:],
                    start=True,
                    stop=True,
                )
            hT = datap.tile([128, 256], f32, tag="hT")
            nc.vector.tensor_copy(hT[:], ph[:])

            # cT[h, j] (bf16) and aT[h, i]+b1 (fp32) for both h-tiles.
            # aTb4 stores each bias column replicated x4 so that every
            # column sits at a 16-byte offset — the VE bf16 dual-op
            # tensor_scalar falls off a microcode cliff (280ns -> 1.2-3.9us)
            # when the per-partition scalar pointer is not 16B-aligned.
            cT = [datap.tile([128, 256], bf16, tag=f"cT{t}", name=f"cT{t}") for t in range(2)]
            aTb4 = [datap.tile([128, 128, 4], f32, tag=f"aTb4{t}", name=f"aTb4{t}") for t in range(2)]
            for t in range(2):
                pc = pcp.tile([128, 256], f32, tag="pc")
                nc.tensor.matmul(
                    pc[:], lhsT=w1ct_sb[:, t, :], rhs=hT[:], start=True, stop=True
                )
                nc.scalar.copy(cT[t][:], pc[:])
                pa = pcp.tile([128, 128], f32, tag="pa")
                nc.tensor.matmul(
                    pa[:], lhsT=w1at_sb[:, t, :], rhs=hT[:, 0:128], start=True, stop=True
                )
                nc.vector.tensor_scalar(
                    aTb4[t][:, :, :],
                    pa[:].broadcast_to([128, 128, 4]),
                    b1t_sb[:, t : t + 1],
                    None,
                    Alu.add,
                )

            # Pairwise: accumulate each 32-i group into its own PSUM bank,
            # drain groups as they complete.  Act tiles split VE/SE/GP.
            pl = None
            for i in range(128):
                g, r = divmod(i, 32)
                if r == 0:
                    pl = plp.tile([32, 256], f32, tag="pl", name=f"pl{g}")
                for t in range(2):
                    idx = 2 * i + t
                    a_col = aTb4[t][:, i, 0:1]
                    if idx % 3 == 1:
                        asb = actp.tile([128, 256], bf16, tag="acts")
                        nc.scalar.activation(asb[:], cT[t][:], Act.Relu, bias=a_col)
                        mv = asb
                    else:
                        av = actp.tile([128, 256], bf16, tag="actv")
                        nc.vector.tensor_scalar(
                            av[:], cT[t][:], a_col, 0.0, Alu.add, Alu.max
                        )
                        mv = av
                    nc.tensor.matmul(
                        pl[:, :],
                        lhsT=w2b_sb[:, t, r, :],
                        rhs=mv[:],
                        start=(r == 0 and t == 0),
                        stop=(r == 31 and t == 1),
                    )
                if r == 31:
                    osb = datap.tile([32, 256], f32, tag=f"osb{g}", name=f"osb{g}")
                    nc.vector.tensor_scalar(
                        osb[:], pl[:, :], b2c_sb[0:32, :], None, Alu.add
                    )
                    nc.sync.dma_start(out=outd[g], in_=osb[:])

    nc.compile()
    return nc


def make_in_maps(nodefeat, W1, b1, W2, b2):
    """Host-side sharding/layout prep."""
    import ml_dtypes

    bf16 = ml_dtypes.bfloat16
    nodefeat = np.asarray(nodefeat, dtype=np.float32)
    W1 = np.asarray(W1, dtype=np.float32)
    b1 = np.asarray(b1, dtype=np.float32)
    W2 = np.asarray(W2, dtype=np.float32)
    b2 = np.asarray(b2, dtype=np.float32)

    smat = (np.repeat(np.eye(8, dtype=np.float32), 16, axis=0) / 16.0).astype(bf16)

    W1a, W1c = W1[:, :_F], W1[:, _F:]
    w1at = np.ascontiguousarray(np.stack([W1a[:128].T, W1a[128:].T], axis=1))
    w1ct = np.ascontiguousarray(np.stack([W1c[:128].T, W1c[128:].T], axis=1))
    b1t = np.ascontiguousarray(b1.reshape(2, 128).T)

    w2r = W2[0].reshape(2, 128)  # [ht, p]
    w2b = np.zeros((128, 2, 32, 32), dtype=np.float32)
    idx = np.arange(32)
    w2b[:, :, idx, idx] = w2r.T[:, :, None]
    w2b = w2b.astype(bf16)

    b2c = np.full((128, 1), b2[0], dtype=np.float32)

    in_maps = []
    for k in range(_NCORES):
        b, ih = divmod(k, 2)
        nf_b = nodefeat[b]  # [256, 16, 128]
        if ih:
            nf_b = np.concatenate([nf_b[128:], nf_b[:128]], axis=0)
        # [256,16,128] -> [32 oct, (j8,t16)=128, 128 f] -> [128, 32, 128]
        nf_dev = np.ascontiguousarray(
            nf_b.reshape(32, 128, 128).transpose(1, 0, 2).astype(bf16)
        )
        in_maps.append(
            {
                "nf": nf_dev,
                "smat": smat,
                "w1at": w1at,
                "w1ct": w1ct,
                "b1t": b1t,
                "w2b": w2b,
                "b2c": b2c,
            }
        )
    return in_maps


def assemble_output(results):
    out = np.empty((_B, _N, _N), dtype=np.float32)
    for k in range(_NCORES):
        b, ih = divmod(k, 2)
        r = results[k]["out"].reshape(128, 256)  # [i, j] (j core-local order)
        if ih:
            r = np.concatenate([r[:, 128:], r[:, :128]], axis=1)
        out[b, ih * 128 : (ih + 1) * 128, :] = r
    return out


def _get_nc():
    if "nc" not in _CACHE:
        _CACHE["nc"] = build_nc()
    return _CACHE["nc"]


def kernel(nodefeat, W1, b1, W2, b2):
    _ensure_paths()
    from concourse.bass_utils import run_bass_kernel_spmd

    nc = _get_nc()
    in_maps = make_in_maps(nodefeat, W1, b1, W2, b2)
    res = run_bass_kernel_spmd(nc, in_maps, list(range(_NCORES)))
    return assemble_output(res.results)
